# revision 1
# baseline (speedup 1.0000x reference)
"""Trainium2 Bass kernel for AdaptiveEdgeGNN (2-layer gated edge conv + pool).

Sharding: edges sorted by dst, cores own equal tile-aligned dst ranges
(NLOC = NPAD/8 nodes each) so aggregates stay core-local (no all-reduce of
node features). Within a core, edges are bucketed by src chunk (4 chunks)
so dma_gather's int16 indices stay in range.

Math: the edge MLP's first layer decomposes as ef@m1w = A[src] + B[dst]
(A = h@m1w[:64], B = h@m1w[64:]), and every node table is affine in the
node input, so one matmul per 128-node tile emits [h | A | B' | 0] rows
(B' folds the edge-MLP bias). Per edge: gather [h|A] by src and [B'] by
dst, z = relu(A+B'), gate = sigmoid(z@m2w + m2b), msg = gate*h, and
dma_scatter_add accumulates msg into the local dst aggregate. x2 = relu(agg1)
is AllGather'd between convs; per-core partial graph pools are AllReduce'd.

The graph is identical on all 8 cores (SPMD); everything per-core comes in
through input tensors (xplus_loc, src16/dst16/batch16 index arrays).
"""
import numpy as np

CFG_REAL = dict(N=100000, E=1600000, G=100, NC=8, BLOCK=4096, NCHUNK=4)


def derive(cfg):
    """Static layout constants shared by host prep and the graph builder."""
    import math
    N, NC, NCHUNK = cfg["N"], cfg["NC"], cfg["NCHUNK"]
    d = dict(cfg)
    unit = 128 * math.lcm(NC, NCHUNK)
    d["NPAD"] = -(-N // unit) * unit
    d["NTILE"] = d["NPAD"] // 128
    d["CTILE"] = d["NTILE"] // NCHUNK
    d["CHUNK"] = d["CTILE"] * 128            # nodes per src chunk
    d["NLOC"] = d["NPAD"] // NC              # dst nodes owned per core
    assert d["NLOC"] % 128 == 0
    d["LTILE"] = d["NLOC"] // 128
    d["DUMP"] = d["NLOC"]                    # dump row (first pad row)
    d["LROWS"] = d["NLOC"] + 128             # agg/Tdst rows incl. dump tile
    assert d["CHUNK"] - 1 < 32768 and d["LROWS"] - 1 < 32768
    d["T"] = cfg["BLOCK"] // 128             # tokens per partition per block
    return d


def wrap_idx_blocks(idx, block):
    """Wrap flat ints (len = nblocks*block) into the DMA idx layout: per
    block, token k -> [k%16, k//16]; blocks concatenated along the free
    axis. The 16-row group is replicated across all 8 GpSimd cores
    (rows 16k..16k+15). Returns [128, len/16] int16."""
    n = len(idx)
    assert n % block == 0
    nb = n // block
    out = np.zeros((16, n // 16), np.int16)
    a = np.asarray(idx).reshape(nb, block)
    k = np.arange(block)
    for b in range(nb):
        out[k % 16, b * (block // 16) + k // 16] = a[b]
    return np.tile(out, (8, 1))


def _to_bf16(a):
    import ml_dtypes
    return np.ascontiguousarray(np.asarray(a, np.float32).astype(ml_dtypes.bfloat16))


def prep_host(inputs, cfg=None):
    """Host-side index/layout prep. Returns (in_maps, meta)."""
    d = derive(cfg or CFG_REAL)
    N, E, G, NC, BLOCK, NCHUNK = (d[k] for k in
                                  ("N", "E", "G", "NC", "BLOCK", "NCHUNK"))
    x = np.asarray(inputs["x"], np.float32)
    ei = np.asarray(inputs["edge_index"])
    batch = np.asarray(inputs["batch"]).astype(np.int64)
    src, dst = ei[0].astype(np.int64), ei[1].astype(np.int64)

    f32 = np.float32
    g = {k: np.asarray(v, f32) for k, v in inputs.items()
         if k not in ("x", "edge_index", "batch")}

    Ma1, Mb1 = g["c1_m1w"][:64], g["c1_m1w"][64:]
    Wbig1 = np.zeros((2, 256), f32)
    Wbig1[0, 0:64] = g["c1_lw"][0]
    Wbig1[0, 64:128] = g["c1_lw"][0] @ Ma1
    Wbig1[0, 128:192] = g["c1_lw"][0] @ Mb1
    Wbig1[1, 0:64] = g["c1_lb"]
    Wbig1[1, 64:128] = g["c1_lb"] @ Ma1
    Wbig1[1, 128:192] = g["c1_lb"] @ Mb1 + g["c1_m1b"]

    Ma2, Mb2 = g["c2_m1w"][:64], g["c2_m1w"][64:]
    W2h = np.vstack([g["c2_lw"], g["c2_lb"][None, :]])   # [65, 64]
    Wbig2 = np.zeros((65, 256), f32)
    Wbig2[:, 0:64] = W2h
    Wbig2[:, 64:128] = W2h @ Ma2
    Wbig2[:, 128:192] = W2h @ Mb2
    Wbig2[64, 128:192] += g["c2_m1b"]

    xplus = np.zeros((2, d["NPAD"]), f32)
    xplus[0, :N] = x[:, 0]
    xplus[1, :N] = 1.0

    cnts = np.bincount(batch, minlength=G).astype(f32)
    inv_cnt = np.zeros((128, 1), f32)
    inv_cnt[:G, 0] = 1.0 / np.maximum(cnts, 1.0)
    headw_tile = np.tile(g["head_w"][:, 0], (128, 1)).astype(f32)

    # ---- edge sharding: dst-sorted, equal tile-aligned dst ranges, then
    # per 128-node dst window, per src chunk, padded to SUBW subtiles ----
    order = np.argsort(dst, kind="stable")
    src_s, dst_s = src[order], dst[order]
    NLOC, CHUNK, LTILE = d["NLOC"], d["CHUNK"], d["LTILE"]
    core_edge = np.searchsorted(dst_s, [NLOC * c for c in range(NC + 1)])

    # windows[c][w][b] = (srcs, dsts) for core c, dst window w, src chunk b
    windows = [[None] * LTILE for _ in range(NC)]
    subw = 1
    for c in range(NC):
        e0, e1 = core_edge[c], core_edge[c + 1]
        s, t = src_s[e0:e1], dst_s[e0:e1] - NLOC * c
        wedge = np.searchsorted(t, [128 * w for w in range(LTILE + 1)])
        for w in range(LTILE):
            sw = s[wedge[w]:wedge[w + 1]]
            tw = t[wedge[w]:wedge[w + 1]]
            ch = sw // CHUNK
            per = []
            for b in range(NCHUNK):
                m = ch == b
                per.append((sw[m], tw[m]))
                subw = max(subw, -(-int(m.sum()) // 128))
            windows[c][w] = per
    d["SUBW"] = subw
    d["WBLK"] = NCHUNK * subw * 128          # tokens per window
    meta = dict(cfg=d,
                m2b1=float(g["c1_m2b"][0]), m2b2=float(g["c2_m2b"][0]),
                head_b=float(g["head_b"][0]))

    WBLK = d["WBLK"]
    SUB = subw * 128
    Twin = WBLK // 128
    m2w_rep1 = np.tile(g["c1_m2w"][:, 0], (128, Twin))
    m2w_rep2 = np.tile(g["c2_m2w"][:, 0], (128, Twin))
    iota128 = np.tile(np.arange(128, dtype=f32), (128, Twin))
    in_maps = []
    for c in range(NC):
        lo = NLOC * c
        dump = d["DUMP"]
        src16_l, dst16_l, dstw_l = [], [], []
        for w in range(LTILE):
            for b in range(NCHUNK):
                sb, tb = windows[c][w][b]
                pad = SUB - len(sb)
                src16_l.append(np.concatenate(
                    [sb - b * CHUNK, np.zeros(pad, np.int64)]))
                dst16_l.append(np.concatenate(
                    [tb, np.full(pad, dump, np.int64)]))
                dstw_l.append(np.concatenate(
                    [tb - 128 * w, np.full(pad, 999, np.int64)]))
        src16 = wrap_idx_blocks(np.concatenate(src16_l), SUB)
        dst16 = wrap_idx_blocks(np.concatenate(dst16_l), WBLK)
        # dstw values in token layout [128, LTILE * T]: token k of window w
        # at [k%128, w*T + k//128]
        T = WBLK // 128
        dstw = np.concatenate(dstw_l).reshape(LTILE, WBLK)
        k = np.arange(WBLK)
        dstwb = np.zeros((128, LTILE * T), np.float32)
        for w in range(LTILE):
            dstwb[k % 128, w * T + k // 128] = dstw[w]

        # batch value per local node, token layout [128, LTILE]
        bl = np.full(NLOC, 999, np.int64)            # sentinel -> no graph
        nreal = max(0, min(N - lo, NLOC))
        if nreal > 0:
            bl[:nreal] = batch[lo:lo + nreal]
        batchb = np.zeros((128, LTILE), np.float32)
        kk = np.arange(NLOC)
        batchb[kk % 128, kk // 128] = bl

        xplus_loc = xplus[:, lo:lo + NLOC].copy()

        in_maps.append({
            "xplus": _to_bf16(xplus), "xplus_loc": _to_bf16(xplus_loc),
            "Wbig1": _to_bf16(Wbig1), "Wbig2": _to_bf16(Wbig2),
            "m2w_rep1": _to_bf16(m2w_rep1), "m2w_rep2": _to_bf16(m2w_rep2),
            "iota128": _to_bf16(iota128),
            "headw": headw_tile, "inv_cnt": inv_cnt,
            "src16": src16, "dst16": dst16,
            "dstwb": _to_bf16(dstwb), "batchb": _to_bf16(batchb),
        })
    return in_maps, meta


def build(meta, debug=False):
    import concourse.bacc as bacc
    import concourse.mybir as mybir
    import concourse.tile as tile

    d = meta["cfg"]
    N, E, G, NC, NCHUNK = (d[k] for k in ("N", "E", "G", "NC", "NCHUNK"))
    NPAD, NTILE, CTILE, CHUNK = d["NPAD"], d["NTILE"], d["CTILE"], d["CHUNK"]
    NLOC, LTILE, LROWS = d["NLOC"], d["LTILE"], d["LROWS"]
    SUBW, WBLK = d["SUBW"], d["WBLK"]
    T = WBLK // 128                       # tokens per partition per window
    SUB = SUBW * 128                      # tokens per (window, chunk)
    F32, BF16, I16 = mybir.dt.float32, mybir.dt.bfloat16, mybir.dt.int16
    AF = mybir.ActivationFunctionType
    OP = mybir.AluOpType

    nc = bacc.Bacc("TRN2", target_bir_lowering=False, debug=False,
                   num_devices=NC)
    xplus = nc.dram_tensor("xplus", [2, NPAD], BF16, kind="ExternalInput")
    xplus_loc = nc.dram_tensor("xplus_loc", [2, NLOC], BF16, kind="ExternalInput")
    Wbig1 = nc.dram_tensor("Wbig1", [2, 256], BF16, kind="ExternalInput")
    Wbig2 = nc.dram_tensor("Wbig2", [65, 256], BF16, kind="ExternalInput")
    m2w_rep1 = nc.dram_tensor("m2w_rep1", [128, T * 64], BF16, kind="ExternalInput")
    m2w_rep2 = nc.dram_tensor("m2w_rep2", [128, T * 64], BF16, kind="ExternalInput")
    iota128 = nc.dram_tensor("iota128", [128, T * 128], BF16, kind="ExternalInput")
    headw = nc.dram_tensor("headw", [128, 64], F32, kind="ExternalInput")
    inv_cnt = nc.dram_tensor("inv_cnt", [128, 1], F32, kind="ExternalInput")
    src16 = nc.dram_tensor("src16", [128, LTILE * WBLK // 16], I16,
                           kind="ExternalInput")
    dst16 = nc.dram_tensor("dst16", [128, LTILE * WBLK // 16], I16,
                           kind="ExternalInput")
    dstwb = nc.dram_tensor("dstwb", [128, LTILE * T], BF16, kind="ExternalInput")
    batchb = nc.dram_tensor("batchb", [128, LTILE], BF16, kind="ExternalInput")
    out = nc.dram_tensor("out", [G, 1], F32, kind="ExternalOutput")

    dbg = dict(kind="ExternalOutput") if debug else {}
    Tsrc1 = nc.dram_tensor("Tsrc1", [NPAD, 128], BF16, **dbg)
    Tdst1 = nc.dram_tensor("Tdst1", [LROWS, 128], BF16, **dbg)
    Tsrc2 = nc.dram_tensor("Tsrc2", [NPAD, 128], BF16)
    Tdst2 = nc.dram_tensor("Tdst2", [LROWS, 128], BF16)
    agg1 = nc.dram_tensor("agg1", [NLOC, 64], F32, **dbg)
    agg2 = nc.dram_tensor("agg2", [NLOC, 64], F32)
    x2loc = nc.dram_tensor("x2loc", [NLOC, 128], BF16)
    x2full = nc.dram_tensor("x2full", [NPAD, 128], BF16, addr_space="Shared")
    x2dbg = nc.dram_tensor("x2dbg", [NPAD, 128], BF16, **dbg) if debug else None
    poolp = nc.dram_tensor("poolp", [128, 64], F32)
    poolf = nc.dram_tensor("poolf", [128, 64], F32, addr_space="Shared")

    with tile.TileContext(nc) as tc:
        with (
            tc.tile_pool(name="const", bufs=1) as constp,
            tc.tile_pool(name="sb", bufs=3) as pool,
            tc.tile_pool(name="ps", bufs=2, space="PSUM") as psum,
        ):
            w1 = constp.tile([2, 256], BF16)
            nc.sync.dma_start(w1[:], Wbig1[:])
            w2 = constp.tile([65, 256], BF16)
            nc.sync.dma_start(w2[:], Wbig2[:])
            mr1 = constp.tile([128, T * 64], BF16)
            nc.sync.dma_start(mr1[:], m2w_rep1[:])
            mr2 = constp.tile([128, T * 64], BF16)
            nc.sync.dma_start(mr2[:], m2w_rep2[:])
            iot = constp.tile([128, T * 128], BF16)
            nc.sync.dma_start(iot[:], iota128[:])
            zf = constp.tile([128, 64], F32)
            nc.gpsimd.memset(zf[:], 0.0)
            zb = constp.tile([128, 128], BF16)
            nc.gpsimd.memset(zb[:], 0.0)

            nc.sync.dma_start(poolp[:], zf[:])
            # Tdst pad tile (dump rows) must be finite for the gathers
            nc.sync.dma_start(Tdst1[NLOC:LROWS, :], zb[:])
            nc.sync.dma_start(Tdst2[NLOC:LROWS, :], zb[:])

            def table_tile(lhs_ap, Wt, psrc, pdst, t):
                """One 128-node tile: matmul -> [h|A|B'|0], write tables."""
                pt = psum.tile([128, 256], F32, tag="ptab")
                nc.tensor.matmul(pt[:], lhs_ap, Wt[:], start=True, stop=True)
                so = pool.tile([128, 256], BF16, tag="so")
                nc.scalar.activation(so[:], pt[:], AF.Copy)
                if psrc is not None:
                    nc.sync.dma_start(psrc[t * 128:(t + 1) * 128, :],
                                      so[:, 0:128])
                if pdst is not None:
                    nc.sync.dma_start(pdst[t * 128:(t + 1) * 128, :],
                                      so[:, 128:256])

            # conv1 tables: global Tsrc1 from xplus, local Tdst1 from xplus_loc
            for t in range(NTILE):
                lhs = pool.tile([2, 128], BF16, tag="lhs1")
                nc.sync.dma_start(lhs[:], xplus[:, t * 128:(t + 1) * 128])
                table_tile(lhs[:], w1, Tsrc1, None, t)
            for t in range(LTILE):
                lhs = pool.tile([2, 128], BF16, tag="lhs1")
                nc.sync.dma_start(lhs[:], xplus_loc[:, t * 128:(t + 1) * 128])
                table_tile(lhs[:], w1, None, Tdst1, t)

            import os
            ABL = set(os.environ.get("KABL", "").split(","))

            def gather_split(out_tile, t0, in_ap, idx_tile, c0, total, elem):
                """dma_gather capped at 1024 idxs/call (HW SWDGE ring limit).
                out_tile [128, T, elem] from token-row t0; idx cols from c0."""
                done = 0
                while done < total:
                    n = min(1024, total - done)
                    tr = t0 + done // 128
                    cc = c0 + done // 16
                    nc.gpsimd.dma_gather(
                        out_ap=out_tile[:, tr:tr + n // 128, :],
                        in_ap=in_ap,
                        idxs_ap=idx_tile[:, cc:cc + n // 16],
                        num_idxs=n, num_idxs_reg=n, elem_size=elem)
                    done += n

            def edge_pipeline(Tsrc, Tdst, mr, m2b, agg):
                for w in range(LTILE):
                    i0 = w * (WBLK // 16)
                    tsi = pool.tile([128, WBLK // 16], I16, tag="tsi")
                    nc.sync.dma_start(tsi[:], src16[:, i0:i0 + WBLK // 16])
                    tdi = pool.tile([128, WBLK // 16], I16, tag="tdi")
                    nc.sync.dma_start(tdi[:], dst16[:, i0:i0 + WBLK // 16])
                    tdw = pool.tile([128, T], BF16, tag="tdw")
                    nc.sync.dma_start(tdw[:], dstwb[:, w * T:(w + 1) * T])
                    gs = pool.tile([128, T, 128], BF16, tag="gs")
                    if "nogather" in ABL:
                        nc.gpsimd.memset(gs[:], 0.125)
                    else:
                        for b in range(NCHUNK):
                            gather_split(
                                gs, b * SUBW,
                                Tsrc[b * CHUNK:(b + 1) * CHUNK, :],
                                tsi, b * (SUB // 16), SUB, 128)
                    gd = pool.tile([128, T, 128], BF16, tag="gd")
                    if "nogather" in ABL:
                        nc.gpsimd.memset(gd[:], 0.125)
                    else:
                        gather_split(gd, 0, Tdst[:], tdi, 0, WBLK, 128)
                    tz = pool.tile([128, T, 64], BF16, tag="tz")
                    nc.vector.tensor_tensor(
                        out=tz[:], in0=gs[:, :, 64:128],
                        in1=gd[:, :, 0:64], op=OP.add)
                    nc.vector.tensor_scalar_max(tz[:], tz[:], 0.0)
                    tzw = pool.tile([128, T, 64], BF16, tag="tzw")
                    nc.vector.tensor_tensor(
                        out=tzw[:], in0=tz[:],
                        in1=mr[:].rearrange("p (t f) -> p t f", t=T),
                        op=OP.mult)
                    graw = pool.tile([128, T], F32, tag="graw")
                    nc.vector.tensor_reduce(
                        out=graw[:], in_=tzw[:], op=OP.add,
                        axis=mybir.AxisListType.X)
                    gate = pool.tile([128, T], BF16, tag="gate")
                    nc.scalar.activation(gate[:], graw[:], AF.Sigmoid,
                                         bias=m2b)
                    tmsg = pool.tile([128, T, 64], BF16, tag="tmsg")
                    nc.vector.tensor_tensor(
                        out=tmsg[:], in0=gs[:, :, 0:64],
                        in1=gate[:].broadcast_to([128, T, 64]),
                        op=OP.mult)
                    # one-hot S[token, j] = (dstw[token] == j), zero for pads
                    S = pool.tile([128, T, 128], BF16, tag="S")
                    nc.vector.tensor_tensor(
                        out=S[:], in0=iot[:].rearrange("p (t f) -> p t f", t=T),
                        in1=tdw[:].broadcast_to([128, T, 128]),
                        op=OP.is_equal)
                    if "nomm" in ABL:
                        ao = pool.tile([128, 64], F32, tag="ao")
                        nc.vector.tensor_reduce(
                            out=ao[:], in_=S[:].rearrange(
                                "p t f -> p (t f)").rearrange(
                                "p (a b) -> p a b", b=64),
                            op=OP.add, axis=mybir.AxisListType.X)
                    else:
                        pw = psum.tile([128, 64], F32, tag="pw")
                        for s in range(T):
                            nc.tensor.matmul(pw[:], S[:, s, :], tmsg[:, s, :],
                                             start=(s == 0), stop=(s == T - 1))
                        ao = pool.tile([128, 64], F32, tag="ao")
                        nc.scalar.activation(ao[:], pw[:], AF.Copy)
                    nc.sync.dma_start(agg[w * 128:(w + 1) * 128, :], ao[:])

            edge_pipeline(Tsrc1, Tdst1, mr1, meta["m2b1"], agg1)

            # x2 = relu(agg1), with ones column at 64, zeros at 65:128
            for t in range(LTILE):
                av = pool.tile([128, 64], F32, tag="av")
                nc.sync.dma_start(av[:], agg1[t * 128:(t + 1) * 128, :])
                xt = pool.tile([128, 128], BF16, tag="xt")
                nc.scalar.activation(xt[:, 0:64], av[:], AF.Relu)
                nc.gpsimd.memset(xt[:, 64:65], 1.0)
                nc.gpsimd.memset(xt[:, 65:128], 0.0)
                nc.sync.dma_start(x2loc[t * 128:(t + 1) * 128, :], xt[:])

            # AllGather x2 across the 8 cores
            nc.gpsimd.collective_compute(
                "AllGather", OP.bypass,
                replica_groups=[list(range(NC))],
                ins=[x2loc[:].opt()], outs=[x2full[:].opt()])

            if x2dbg is not None:
                for t in range(NTILE):
                    tv = pool.tile([128, 128], BF16, tag="tv")
                    nc.sync.dma_start(tv[:], x2full[t * 128:(t + 1) * 128, :])
                    nc.sync.dma_start(x2dbg[t * 128:(t + 1) * 128, :], tv[:])

            # conv2 tables: Tsrc2 from x2full (global), Tdst2 from x2loc
            for t in range(NTILE):
                lhs = pool.tile([128, 128], BF16, tag="lhs2")
                nc.sync.dma_start(lhs[:], x2full[t * 128:(t + 1) * 128, :],
                                  transpose=True)
                table_tile(lhs[0:65, :], w2, Tsrc2, None, t)
            for t in range(LTILE):
                lhs = pool.tile([128, 128], BF16, tag="lhs2")
                nc.sync.dma_start(lhs[:], x2loc[t * 128:(t + 1) * 128, :],
                                  transpose=True)
                table_tile(lhs[0:65, :], w2, None, Tdst2, t)

            edge_pipeline(Tsrc2, Tdst2, mr2, meta["m2b2"], agg2)

            # pooling: pool[g] += sum_nodes relu(agg2) via one-hot matmuls
            tbb = pool.tile([128, LTILE], BF16, tag="tbb")
            nc.sync.dma_start(tbb[:], batchb[:])
            pp = psum.tile([128, 64], F32, tag="pp")
            for t in range(LTILE):
                av = pool.tile([128, 64], F32, tag="av")
                nc.sync.dma_start(av[:], agg2[t * 128:(t + 1) * 128, :])
                h2t = pool.tile([128, 64], BF16, tag="h2t")
                nc.scalar.activation(h2t[:], av[:], AF.Relu)
                Sp = pool.tile([128, 128], BF16, tag="Sp")
                nc.vector.tensor_tensor(
                    out=Sp[:], in0=iot[:, 0:128],
                    in1=tbb[:, t:t + 1].broadcast_to([128, 128]),
                    op=OP.is_equal)
                nc.tensor.matmul(pp[:], Sp[:], h2t[:],
                                 start=(t == 0), stop=(t == LTILE - 1))
            pps = pool.tile([128, 64], F32, tag="pps")
            nc.scalar.activation(pps[:], pp[:], AF.Copy)
            nc.sync.dma_start(poolp[0:G, :], pps[0:G, :])

            nc.gpsimd.collective_compute(
                "AllReduce", OP.add,
                replica_groups=[list(range(NC))],
                ins=[poolp[:].opt()], outs=[poolf[:].opt()])

            # head: out = (pool/cnt) @ head_w + head_b
            pf = pool.tile([128, 64], F32)
            nc.sync.dma_start(pf[:], poolf[:])
            ic = pool.tile([128, 1], F32)
            nc.sync.dma_start(ic[:], inv_cnt[:])
            hw = pool.tile([128, 64], F32)
            nc.sync.dma_start(hw[:], headw[:])
            pm = pool.tile([128, 64], F32)
            nc.vector.tensor_scalar(pm[:], pf[:], ic[:], None, op0=OP.mult)
            ph = pool.tile([128, 64], F32)
            nc.vector.tensor_tensor(out=ph[:], in0=pm[:], in1=hw[:],
                                    op=OP.mult)
            po = pool.tile([128, 1], F32)
            nc.vector.tensor_reduce(out=po[:], in_=ph[:], op=OP.add,
                                    axis=mybir.AxisListType.X)
            pb = pool.tile([128, 1], F32)
            nc.vector.tensor_scalar_add(pb[:], po[:], meta["head_b"])
            nc.sync.dma_start(out[:], pb[0:G, :])

    nc.finalize()
    return nc


_CACHE = {}


def kernel(**inputs):
    from concourse.bass_utils import run_bass_kernel_spmd
    in_maps, meta = prep_host(inputs)
    key = "real"
    if key not in _CACHE:
        _CACHE[key] = build(meta)
    nc = _CACHE[key]
    res = run_bass_kernel_spmd(nc, in_maps, core_ids=list(range(meta["cfg"]["NC"])))
    return np.asarray(res.results[0]["out"], np.float32)

